# revision 19
# baseline (speedup 1.0000x reference)
"""Trainium2 kernel for running-average pooling with cached state.

Math (per batch n):
  G[t] = cached_len*cached_avg + cumsum(x[:, n, :], axis=0)[t]
  y[t] = G[t] / (t + 1 + cached_len)
  new_cached_len = cached_len + T ; new_cached_avg = y[T-1]

Sharding: data-parallel over N=16 batches -> 2 batches per core on 8 cores.

Per-core algorithm: blocked cumsum along T via one triangular matmul per
126-row tile per batch. The stationary matrix M1 is triu(ones(127,127)) with
its first column set to all-ones, and the moving tile holds the running carry
row at partition 0 with 126 x-rows at partitions 1..126:
  psum[m] = carry + sum(x_rows[0:m])   (m = 1..126  -> outputs)
  psum[0] = carry + sum(all 126 rows)  (= carry for the next tile)
The carry chains tile-to-tile through an aligned partition-0 ACT copy.
Outputs are scaled by precomputed reciprocals 1/(126*i + m + len) on DVE.

TR=126 (not 127) is critical for DMA speed: the DGE only distributes a
descriptor pattern across its 16 DMA engines for certain inner counts
(126/112/64/128 spread; 127 pins all descriptors on ONE engine at
~156ns/descriptor = 26GB/s). The whole 16MB x shard is loaded up-front
into one resident SBUF tile by 4 dependency-free gpsimd dma_starts
(~200GB/s measured); stores ride the sync HW DGE queue.
"""

import numpy as np

T, N_FULL_BATCH, C = 4096, 16, 512
NB = 2        # batches per core
CC = NB * C   # elements per T-row in per-core DRAM shard
NCORES = 8
TR = 126      # x-rows per full tile
NFULL = 32    # number of full tiles
LASTR = T - TR * NFULL  # 64
GROUP = 2     # tiles per output store group
NG = 16       # full groups (NG * GROUP == NFULL)
NTILES = NFULL + 1
CHUNKS = [1, 1, 2, 4, 4, 4, 8, 8]  # staggered dep-free load chunk sizes (tiles)

_cached_nc = None


def _build():
    from contextlib import ExitStack

    import concourse.bass as bass
    import concourse.bacc as bacc
    import concourse.tile as tile
    from concourse import mybir

    f32 = mybir.dt.float32
    i32 = mybir.dt.int32

    nc = bacc.Bacc(None, target_bir_lowering=False)
    x_h = nc.declare_dram_parameter("x", [T, NB, C], f32, isOutput=False)
    len_h = nc.declare_dram_parameter("cached_len", [NB], i32, isOutput=False)
    avg_h = nc.declare_dram_parameter("cached_avg", [NB, C], f32, isOutput=False)
    y_h = nc.declare_dram_parameter("y", [T, NB, C], f32, isOutput=True)

    m1_np = np.zeros((128, 128), dtype=np.float32)
    m1_np[0 : TR + 1, 0 : TR + 1] = np.triu(np.ones((TR + 1, TR + 1), dtype=np.float32))
    m1_np[0 : TR + 1, 0] = 1.0
    grid_np = (
        np.arange(NTILES, dtype=np.float32)[None, :] * TR
        + np.arange(128, dtype=np.float32)[:, None]
    )
    grid_np[0, :] = 1.0  # row 0 is never an output; avoid 1/0
    m1_d = nc.inline_tensor(m1_np, name="m1c")
    grid_d = nc.inline_tensor(grid_np, name="gridc")

    ROWS_G = TR * GROUP  # 504 T-rows per group

    def group_ap(h, g):
        full = h[:]
        return bass.AP(
            tensor=full.tensor,
            offset=ROWS_G * g * CC,
            ap=[[CC, TR], [TR * CC, GROUP], [1, CC]],
        )

    with ExitStack() as ctx:
        tc = ctx.enter_context(tile.TileContext(nc))
        sing = ctx.enter_context(tc.tile_pool(name="sing", bufs=1))
        yp = ctx.enter_context(tc.tile_pool(name="yp", bufs=2))
        psp = ctx.enter_context(tc.tile_pool(name="psp", bufs=6, space="PSUM"))

        # Entire x shard resident: tile i at free index i, rows at parts 1..126.
        xall = sing.tile([128, NTILES, CC], f32, name="xall")
        xfull = x_h[:]
        i0 = 0
        for kg in CHUNKS:
            nc.gpsimd.dma_start(
                out=xall[1 : TR + 1, i0 : i0 + kg, :],
                in_=bass.AP(
                    tensor=xfull.tensor,
                    offset=i0 * TR * CC,
                    ap=[[CC, TR], [TR * CC, kg], [1, CC]],
                ),
            )
            i0 += kg
        assert i0 == NFULL
        nc.gpsimd.dma_start(
            out=xall[1 : 1 + LASTR, NFULL, :], in_=x_h[TR * NFULL : T]
        )

        m1 = sing.tile([128, 128], f32, name="m1")
        nc.sync.dma_start(out=m1[:], in_=m1_d[:])
        grid = sing.tile([128, NTILES], f32, name="grid")
        nc.sync.dma_start(out=grid[:], in_=grid_d[:])

        len_i = sing.tile([128, NB], i32, name="len_i")
        lsrc = len_h[:]
        nc.sync.dma_start(
            out=len_i[:],
            in_=bass.AP(tensor=lsrc.tensor, offset=0, ap=[[0, 128], [1, NB]]),
        )
        len_f = sing.tile([128, NB], f32, name="len_f")
        nc.vector.tensor_copy(len_f[:], len_i[:])

        counts = sing.tile([128, NB, NTILES], f32, name="counts")
        recip = sing.tile([128, NB, NTILES], f32, name="recip")
        for n in range(NB):
            nc.vector.tensor_scalar_add(counts[:, n, :], grid[:], len_f[:, n : n + 1])
        nc.vector.reciprocal(recip[:], counts[:])

        # base carries = cached_avg * len at partition 0 of tile 0
        nc.sync.dma_start(out=xall[0:1, 0, :], in_=avg_h[:])
        for n in range(NB):
            nc.vector.tensor_scalar_mul(
                xall[0:1, 0, n * C : (n + 1) * C],
                xall[0:1, 0, n * C : (n + 1) * C],
                len_f[0:1, n : n + 1],
            )

        P = TR + 1  # 127 active partitions in compute
        for g in range(NG):
            yb = yp.tile([128, GROUP, CC], f32, name="yb")
            for j in range(GROUP):
                i = GROUP * g + j
                for n in range(NB):
                    ps = psp.tile([128, C], f32, name="ps")
                    nc.tensor.matmul(
                        out=ps[0:P, :],
                        lhsT=m1[0:P, 0:P],
                        rhs=xall[0:P, i, n * C : (n + 1) * C],
                        start=True,
                        stop=True,
                    )
                    nc.scalar.activation(
                        out=xall[0:1, i + 1, n * C : (n + 1) * C],
                        in_=ps[0:1, :],
                        func=mybir.ActivationFunctionType.Copy,
                    )
                    nc.vector.tensor_scalar_mul(
                        yb[0:P, j, n * C : (n + 1) * C],
                        ps[0:P, :],
                        recip[0:P, n, i : i + 1],
                    )
            nc.sync.dma_start(out=group_ap(y_h, g), in_=yb[1 : TR + 1, :, :])

        # last (short) tile: 64 rows, carry already at xall[0:1, NFULL, :]
        PL = 1 + LASTR
        ybl = yp.tile([PL, CC], f32, name="ybl", bufs=1)
        for n in range(NB):
            ps = psp.tile([128, C], f32, name="ps")
            nc.tensor.matmul(
                out=ps[0:PL, :],
                lhsT=m1[0:PL, 0:PL],
                rhs=xall[0:PL, NFULL, n * C : (n + 1) * C],
                start=True,
                stop=True,
            )
            nc.vector.tensor_scalar_mul(
                ybl[0:PL, n * C : (n + 1) * C],
                ps[0:PL, :],
                recip[0:PL, n, NFULL : NFULL + 1],
            )
        nc.sync.dma_start(out=y_h[TR * NFULL : T], in_=ybl[1:PL, :])

    nc.finalize()
    return nc


def _get_nc():
    global _cached_nc
    if _cached_nc is None:
        _cached_nc = _build()
    return _cached_nc


def kernel(x, cached_len, cached_avg, _trace=False):
    from concourse.bass_utils import run_bass_kernel_spmd

    x = np.asarray(x, dtype=np.float32)
    cached_len = np.asarray(cached_len, dtype=np.int32)
    cached_avg = np.asarray(cached_avg, dtype=np.float32)

    nc = _get_nc()
    in_maps = []
    for c in range(NCORES):
        lo, hi = NB * c, NB * (c + 1)
        in_maps.append(
            {
                "x": np.ascontiguousarray(x[:, lo:hi, :]),
                "cached_len": np.ascontiguousarray(cached_len[lo:hi]),
                "cached_avg": np.ascontiguousarray(cached_avg[lo:hi, :]),
            }
        )
    res = run_bass_kernel_spmd(nc, in_maps, core_ids=list(range(NCORES)), trace=_trace)
    new_x = np.concatenate([res.results[c]["y"] for c in range(NCORES)], axis=1)
    new_cached_len = cached_len + T
    new_cached_avg = new_x[-1].copy()
    if _trace:
        return (new_x, new_cached_len, new_cached_avg), res
    return new_x, new_cached_len, new_cached_avg


# revision 20
# speedup vs baseline: 1.1629x; 1.1629x over previous
"""Trainium2 kernel for running-average pooling with cached state.

Math (per batch n):
  G[t] = cached_len*cached_avg + cumsum(x[:, n, :], axis=0)[t]
  y[t] = G[t] / (t + 1 + cached_len)
  new_cached_len = cached_len + T ; new_cached_avg = y[T-1]

Sharding: data-parallel over N=16 batches -> 2 batches per core on 8 cores.

Per-core algorithm: blocked cumsum along T via one triangular matmul per
126-row tile per batch. The stationary matrix M1 is triu(ones(127,127)) with
its first column set to all-ones, and the moving tile holds the running carry
row at partition 0 with 126 x-rows at partitions 1..126:
  psum[m] = carry + sum(x_rows[0:m])   (m = 1..126  -> outputs)
  psum[0] = carry + sum(all 126 rows)  (= carry for the next tile)
The carry chains tile-to-tile through an aligned partition-0 ACT copy.
Outputs are scaled by precomputed reciprocals 1/(126*i + m + len) on DVE.

TR=126 (not 127) is critical for DMA speed: the DGE only distributes a
descriptor pattern across its 16 DMA engines for certain inner counts
(126/112/64/128 spread; 127 pins all descriptors on ONE engine at
~156ns/descriptor = 26GB/s). The whole 16MB x shard is loaded up-front
into one resident SBUF tile by 4 dependency-free gpsimd dma_starts
(~200GB/s measured); stores ride the sync HW DGE queue.
"""

import numpy as np

T, N_FULL_BATCH, C = 4096, 16, 512
NB = 2        # batches per core
CC = NB * C   # elements per T-row in per-core DRAM shard
NCORES = 8
TR = 126      # x-rows per full tile
NFULL = 32    # number of full tiles
LASTR = T - TR * NFULL  # 64
GROUP = 4     # tiles per output store group
NG = 8        # full groups (NG * GROUP == NFULL)
NTILES = NFULL + 1
CHUNKS = [8, 8, 8, 8]  # dep-free load chunk sizes (tiles)

_cached_nc = None


def _build():
    from contextlib import ExitStack

    import concourse.bass as bass
    import concourse.bacc as bacc
    import concourse.tile as tile
    from concourse import mybir

    f32 = mybir.dt.float32
    i32 = mybir.dt.int32

    nc = bacc.Bacc(None, target_bir_lowering=False)
    x_h = nc.declare_dram_parameter("x", [T, NB, C], f32, isOutput=False)
    len_h = nc.declare_dram_parameter("cached_len", [NB], i32, isOutput=False)
    avg_h = nc.declare_dram_parameter("cached_avg", [NB, C], f32, isOutput=False)
    y_h = nc.declare_dram_parameter("y", [T, NB, C], f32, isOutput=True)

    m1_np = np.zeros((128, 128), dtype=np.float32)
    m1_np[0 : TR + 1, 0 : TR + 1] = np.triu(np.ones((TR + 1, TR + 1), dtype=np.float32))
    m1_np[0 : TR + 1, 0] = 1.0
    grid_np = (
        np.arange(NTILES, dtype=np.float32)[None, :] * TR
        + np.arange(128, dtype=np.float32)[:, None]
    )
    grid_np[0, :] = 1.0  # row 0 is never an output; avoid 1/0
    m1_d = nc.inline_tensor(m1_np, name="m1c")
    grid_d = nc.inline_tensor(grid_np, name="gridc")

    ROWS_G = TR * GROUP  # 504 T-rows per group

    def group_ap(h, g):
        full = h[:]
        return bass.AP(
            tensor=full.tensor,
            offset=ROWS_G * g * CC,
            ap=[[CC, TR], [TR * CC, GROUP], [1, CC]],
        )

    with ExitStack() as ctx:
        tc = ctx.enter_context(tile.TileContext(nc))
        sing = ctx.enter_context(tc.tile_pool(name="sing", bufs=1))
        yp = ctx.enter_context(tc.tile_pool(name="yp", bufs=2))
        psp = ctx.enter_context(tc.tile_pool(name="psp", bufs=6, space="PSUM"))

        # Entire x shard resident: tile i at free index i, rows at parts 1..126.
        xall = sing.tile([128, NTILES, CC], f32, name="xall")
        xfull = x_h[:]
        i0 = 0
        for kg in CHUNKS:
            nc.gpsimd.dma_start(
                out=xall[1 : TR + 1, i0 : i0 + kg, :],
                in_=bass.AP(
                    tensor=xfull.tensor,
                    offset=i0 * TR * CC,
                    ap=[[CC, TR], [TR * CC, kg], [1, CC]],
                ),
            )
            i0 += kg
        assert i0 == NFULL
        nc.gpsimd.dma_start(
            out=xall[1 : 1 + LASTR, NFULL, :], in_=x_h[TR * NFULL : T]
        )

        m1 = sing.tile([128, 128], f32, name="m1")
        nc.sync.dma_start(out=m1[:], in_=m1_d[:])
        grid = sing.tile([128, NTILES], f32, name="grid")
        nc.sync.dma_start(out=grid[:], in_=grid_d[:])

        len_i = sing.tile([128, NB], i32, name="len_i")
        lsrc = len_h[:]
        nc.sync.dma_start(
            out=len_i[:],
            in_=bass.AP(tensor=lsrc.tensor, offset=0, ap=[[0, 128], [1, NB]]),
        )
        len_f = sing.tile([128, NB], f32, name="len_f")
        nc.vector.tensor_copy(len_f[:], len_i[:])

        counts = sing.tile([128, NB, NTILES], f32, name="counts")
        recip = sing.tile([128, NB, NTILES], f32, name="recip")
        for n in range(NB):
            nc.vector.tensor_scalar_add(counts[:, n, :], grid[:], len_f[:, n : n + 1])
        nc.vector.reciprocal(recip[:], counts[:])

        # base carries = cached_avg * len at partition 0 of tile 0
        nc.sync.dma_start(out=xall[0:1, 0, :], in_=avg_h[:])
        for n in range(NB):
            nc.vector.tensor_scalar_mul(
                xall[0:1, 0, n * C : (n + 1) * C],
                xall[0:1, 0, n * C : (n + 1) * C],
                len_f[0:1, n : n + 1],
            )

        P = TR + 1  # 127 active partitions in compute
        for g in range(NG):
            yb = yp.tile([128, GROUP, CC], f32, name="yb")
            for j in range(GROUP):
                i = GROUP * g + j
                for n in range(NB):
                    ps = psp.tile([128, C], f32, name="ps")
                    nc.tensor.matmul(
                        out=ps[0:P, :],
                        lhsT=m1[0:P, 0:P],
                        rhs=xall[0:P, i, n * C : (n + 1) * C],
                        start=True,
                        stop=True,
                    )
                    nc.scalar.activation(
                        out=xall[0:1, i + 1, n * C : (n + 1) * C],
                        in_=ps[0:1, :],
                        func=mybir.ActivationFunctionType.Copy,
                    )
                    nc.vector.tensor_scalar_mul(
                        yb[0:P, j, n * C : (n + 1) * C],
                        ps[0:P, :],
                        recip[0:P, n, i : i + 1],
                    )
            nc.sync.dma_start(out=group_ap(y_h, g), in_=yb[1 : TR + 1, :, :])

        # last (short) tile: 64 rows, carry already at xall[0:1, NFULL, :]
        PL = 1 + LASTR
        ybl = yp.tile([PL, CC], f32, name="ybl", bufs=1)
        for n in range(NB):
            ps = psp.tile([128, C], f32, name="ps")
            nc.tensor.matmul(
                out=ps[0:PL, :],
                lhsT=m1[0:PL, 0:PL],
                rhs=xall[0:PL, NFULL, n * C : (n + 1) * C],
                start=True,
                stop=True,
            )
            nc.vector.tensor_scalar_mul(
                ybl[0:PL, n * C : (n + 1) * C],
                ps[0:PL, :],
                recip[0:PL, n, NFULL : NFULL + 1],
            )
        nc.sync.dma_start(out=y_h[TR * NFULL : T], in_=ybl[1:PL, :])

    nc.finalize()
    return nc


def _get_nc():
    global _cached_nc
    if _cached_nc is None:
        _cached_nc = _build()
    return _cached_nc


def kernel(x, cached_len, cached_avg, _trace=False):
    from concourse.bass_utils import run_bass_kernel_spmd

    x = np.asarray(x, dtype=np.float32)
    cached_len = np.asarray(cached_len, dtype=np.int32)
    cached_avg = np.asarray(cached_avg, dtype=np.float32)

    nc = _get_nc()
    in_maps = []
    for c in range(NCORES):
        lo, hi = NB * c, NB * (c + 1)
        in_maps.append(
            {
                "x": np.ascontiguousarray(x[:, lo:hi, :]),
                "cached_len": np.ascontiguousarray(cached_len[lo:hi]),
                "cached_avg": np.ascontiguousarray(cached_avg[lo:hi, :]),
            }
        )
    res = run_bass_kernel_spmd(nc, in_maps, core_ids=list(range(NCORES)), trace=_trace)
    new_x = np.concatenate([res.results[c]["y"] for c in range(NCORES)], axis=1)
    new_cached_len = cached_len + T
    new_cached_avg = new_x[-1].copy()
    if _trace:
        return (new_x, new_cached_len, new_cached_avg), res
    return new_x, new_cached_len, new_cached_avg


# revision 21
# speedup vs baseline: 1.2032x; 1.0347x over previous
"""Trainium2 kernel for running-average pooling with cached state.

Math (per batch n):
  G[t] = cached_len*cached_avg + cumsum(x[:, n, :], axis=0)[t]
  y[t] = G[t] / (t + 1 + cached_len)
  new_cached_len = cached_len + T ; new_cached_avg = y[T-1]

Sharding: data-parallel over N=16 batches -> 2 batches per core on 8 cores.

Per-core algorithm: blocked cumsum along T via one triangular matmul per
126-row tile per batch. The stationary matrix M1 is triu(ones(127,127)) with
its first column set to all-ones, and the moving tile holds the running carry
row at partition 0 with 126 x-rows at partitions 1..126:
  psum[m] = carry + sum(x_rows[0:m])   (m = 1..126  -> outputs)
  psum[0] = carry + sum(all 126 rows)  (= carry for the next tile)
The carry chains tile-to-tile through an aligned partition-0 ACT copy.
Outputs are scaled by precomputed reciprocals 1/(126*i + m + len) on DVE.

TR=126 (not 127) is critical for DMA speed: the DGE only distributes a
descriptor pattern across its 16 DMA engines for certain inner counts
(126/112/64/128 spread; 127 pins all descriptors on ONE engine at
~156ns/descriptor = 26GB/s). The whole 16MB x shard is loaded up-front
into one resident SBUF tile by 4 dependency-free gpsimd dma_starts
(~200GB/s measured); stores ride the sync HW DGE queue.
"""

import numpy as np

T, N_FULL_BATCH, C = 4096, 16, 512
NB = 2        # batches per core
CC = NB * C   # elements per T-row in per-core DRAM shard
NCORES = 8
TR = 126      # x-rows per full tile
NFULL = 32    # number of full tiles
LASTR = T - TR * NFULL  # 64
GROUP = 4     # tiles per output store group
NG = 8        # full groups (NG * GROUP == NFULL)
NTILES = NFULL + 1
CHUNKS = [2, 3, 5, 6, 8, 8]  # staggered dep-free load chunk sizes (tiles)

_cached_nc = None


def _build():
    from contextlib import ExitStack

    import concourse.bass as bass
    import concourse.bacc as bacc
    import concourse.tile as tile
    from concourse import mybir

    f32 = mybir.dt.float32
    i32 = mybir.dt.int32

    nc = bacc.Bacc(None, target_bir_lowering=False)
    x_h = nc.declare_dram_parameter("x", [T, NB, C], f32, isOutput=False)
    len_h = nc.declare_dram_parameter("cached_len", [NB], i32, isOutput=False)
    avg_h = nc.declare_dram_parameter("cached_avg", [NB, C], f32, isOutput=False)
    y_h = nc.declare_dram_parameter("y", [T, NB, C], f32, isOutput=True)

    m1_np = np.zeros((128, 128), dtype=np.float32)
    m1_np[0 : TR + 1, 0 : TR + 1] = np.triu(np.ones((TR + 1, TR + 1), dtype=np.float32))
    m1_np[0 : TR + 1, 0] = 1.0
    grid_np = (
        np.arange(NTILES, dtype=np.float32)[None, :] * TR
        + np.arange(128, dtype=np.float32)[:, None]
    )
    grid_np[0, :] = 1.0  # row 0 is never an output; avoid 1/0
    m1_d = nc.inline_tensor(m1_np, name="m1c")
    grid_d = nc.inline_tensor(grid_np, name="gridc")

    ROWS_G = TR * GROUP  # 504 T-rows per group

    def group_ap(h, g):
        full = h[:]
        return bass.AP(
            tensor=full.tensor,
            offset=ROWS_G * g * CC,
            ap=[[CC, TR], [TR * CC, GROUP], [1, CC]],
        )

    with ExitStack() as ctx:
        tc = ctx.enter_context(tile.TileContext(nc))
        sing = ctx.enter_context(tc.tile_pool(name="sing", bufs=1))
        yp = ctx.enter_context(tc.tile_pool(name="yp", bufs=2))
        psp = ctx.enter_context(tc.tile_pool(name="psp", bufs=6, space="PSUM"))

        # Entire x shard resident: tile i at free index i, rows at parts 1..126.
        xall = sing.tile([128, NTILES, CC], f32, name="xall")
        xfull = x_h[:]
        i0 = 0
        for kg in CHUNKS:
            nc.gpsimd.dma_start(
                out=xall[1 : TR + 1, i0 : i0 + kg, :],
                in_=bass.AP(
                    tensor=xfull.tensor,
                    offset=i0 * TR * CC,
                    ap=[[CC, TR], [TR * CC, kg], [1, CC]],
                ),
            )
            i0 += kg
        assert i0 == NFULL
        nc.gpsimd.dma_start(
            out=xall[1 : 1 + LASTR, NFULL, :], in_=x_h[TR * NFULL : T]
        )

        m1 = sing.tile([128, 128], f32, name="m1")
        nc.sync.dma_start(out=m1[:], in_=m1_d[:])
        grid = sing.tile([128, NTILES], f32, name="grid")
        nc.sync.dma_start(out=grid[:], in_=grid_d[:])

        len_i = sing.tile([128, NB], i32, name="len_i")
        lsrc = len_h[:]
        nc.sync.dma_start(
            out=len_i[:],
            in_=bass.AP(tensor=lsrc.tensor, offset=0, ap=[[0, 128], [1, NB]]),
        )
        len_f = sing.tile([128, NB], f32, name="len_f")
        nc.vector.tensor_copy(len_f[:], len_i[:])

        counts = sing.tile([128, NB, NTILES], f32, name="counts")
        recip = sing.tile([128, NB, NTILES], f32, name="recip")
        for n in range(NB):
            nc.vector.tensor_scalar_add(counts[:, n, :], grid[:], len_f[:, n : n + 1])
        nc.vector.reciprocal(recip[:], counts[:])

        # base carries = cached_avg * len at partition 0 of tile 0
        nc.sync.dma_start(out=xall[0:1, 0, :], in_=avg_h[:])
        for n in range(NB):
            nc.vector.tensor_scalar_mul(
                xall[0:1, 0, n * C : (n + 1) * C],
                xall[0:1, 0, n * C : (n + 1) * C],
                len_f[0:1, n : n + 1],
            )

        P = TR + 1  # 127 active partitions in compute
        for g in range(NG):
            yb = yp.tile([128, GROUP, CC], f32, name="yb")
            for j in range(GROUP):
                i = GROUP * g + j
                for n in range(NB):
                    ps = psp.tile([128, C], f32, name="ps")
                    nc.tensor.matmul(
                        out=ps[0:P, :],
                        lhsT=m1[0:P, 0:P],
                        rhs=xall[0:P, i, n * C : (n + 1) * C],
                        start=True,
                        stop=True,
                    )
                    nc.scalar.activation(
                        out=xall[0:1, i + 1, n * C : (n + 1) * C],
                        in_=ps[0:1, :],
                        func=mybir.ActivationFunctionType.Copy,
                    )
                    nc.vector.tensor_scalar_mul(
                        yb[0:P, j, n * C : (n + 1) * C],
                        ps[0:P, :],
                        recip[0:P, n, i : i + 1],
                    )
            nc.sync.dma_start(out=group_ap(y_h, g), in_=yb[1 : TR + 1, :, :])

        # last (short) tile: 64 rows, carry already at xall[0:1, NFULL, :]
        PL = 1 + LASTR
        ybl = yp.tile([PL, CC], f32, name="ybl", bufs=1)
        for n in range(NB):
            ps = psp.tile([128, C], f32, name="ps")
            nc.tensor.matmul(
                out=ps[0:PL, :],
                lhsT=m1[0:PL, 0:PL],
                rhs=xall[0:PL, NFULL, n * C : (n + 1) * C],
                start=True,
                stop=True,
            )
            nc.vector.tensor_scalar_mul(
                ybl[0:PL, n * C : (n + 1) * C],
                ps[0:PL, :],
                recip[0:PL, n, NFULL : NFULL + 1],
            )
        nc.sync.dma_start(out=y_h[TR * NFULL : T], in_=ybl[1:PL, :])

    nc.finalize()
    return nc


def _get_nc():
    global _cached_nc
    if _cached_nc is None:
        _cached_nc = _build()
    return _cached_nc


def kernel(x, cached_len, cached_avg, _trace=False):
    from concourse.bass_utils import run_bass_kernel_spmd

    x = np.asarray(x, dtype=np.float32)
    cached_len = np.asarray(cached_len, dtype=np.int32)
    cached_avg = np.asarray(cached_avg, dtype=np.float32)

    nc = _get_nc()
    in_maps = []
    for c in range(NCORES):
        lo, hi = NB * c, NB * (c + 1)
        in_maps.append(
            {
                "x": np.ascontiguousarray(x[:, lo:hi, :]),
                "cached_len": np.ascontiguousarray(cached_len[lo:hi]),
                "cached_avg": np.ascontiguousarray(cached_avg[lo:hi, :]),
            }
        )
    res = run_bass_kernel_spmd(nc, in_maps, core_ids=list(range(NCORES)), trace=_trace)
    new_x = np.concatenate([res.results[c]["y"] for c in range(NCORES)], axis=1)
    new_cached_len = cached_len + T
    new_cached_avg = new_x[-1].copy()
    if _trace:
        return (new_x, new_cached_len, new_cached_avg), res
    return new_x, new_cached_len, new_cached_avg
